# revision 1
# baseline (speedup 1.0000x reference)
# Trainium2 Bass kernel for nn_BDHBlock (dense transformer block).
#
# Strategy (8 NeuronCores, one shared SPMD program):
#   - Token-parallel for all token-local stages: core c owns flat tokens
#     [512c, 512c+512) of x.reshape(4096, 1024). LayerNorms, the masked
#     sparse linear, QKV / output projections and the FFN are computed
#     locally with replicated (host pre-transposed) weights.
#   - Attention is head-parallel: an AllToAll reshards q/k/v from
#     token-sharded to head-sharded (2 heads x full 4096-token sequence per
#     core), each core runs exact-causal relu attention for its 2 heads,
#     and a second AllToAll reshards the context back to token-sharded.
#     This keeps the program identical on every core (static loops).
#   - Matmul dtypes: float32r (full-rate fp32) for all weight-stationary
#     linears; fp16 for attention and ff2 (w2 cast on-chip after a f32 load).
import numpy as np

import concourse.bass as bass
import concourse.mybir as mybir
import concourse.tile as tile
from concourse import bacc
from concourse.masks import make_identity

B, S, H, NH = 2, 2048, 1024, 16
D = H // NH            # 64
FF = 4 * H             # 4096
NC = 8                 # cores
T = B * S // NC        # 512 tokens per core
TT = T // 128          # 4 token tiles
KT = H // 128          # 8 feature tiles
HPC = 2                # heads per core
F32, F32R, F16 = mybir.dt.float32, mybir.dt.float32r, mybir.dt.float16
ADD, SUB, MUL, MAX = (mybir.AluOpType.add, mybir.AluOpType.subtract,
                      mybir.AluOpType.mult, mybir.AluOpType.max)
AF = mybir.ActivationFunctionType
RG = [list(range(NC))]
EPS = 1e-5

_CACHE = {}


def _r(ap):
    return ap.bitcast(F32R)


def _build():
    nc = bacc.Bacc("TRN2", target_bir_lowering=False, debug=False,
                   num_devices=NC)

    # ---------------- I/O ----------------
    def inp(name, shape, dtype=F32):
        return nc.dram_tensor(name, list(shape), dtype, kind="ExternalInput")

    x_io = inp("x_c", (T, H))
    sfwT_io = inp("sfwT", (H, H))
    maskT_io = inp("maskT", (H, H))
    wT_io = {k: inp(k, (H, H)) for k in ("wqT", "wkT", "wvT", "woT")}
    w1T_io = inp("w1T", (H, FF))
    w2T_io = inp("w2T", (FF, H))
    b_io = {k: inp(k, (H,)) for k in ("sf_b", "bq", "bk", "bv", "bo", "ff2_b")}
    ff1b_io = inp("ff1_b", (FF,))
    gb_io = {k: inp(k, (H,)) for k in ("g1", "b1", "g2", "b2", "g3", "b3")}
    tri_io = inp("tri", (2, 128, 256))           # fp32 diag masks
    bqk_col_io = inp("bqk_col", (128, 2 * KT))   # [p, 2*kt]: bq/bk per-partition cols
    ff1b_col_io = inp("ff1b_col", (128, FF // 128))
    out_io = nc.dram_tensor("out_c", [T, H], F32, kind="ExternalOutput")

    # internal DRAM for collectives (HBM bounce; out must be Shared)
    SLOT = 128 * T                               # elements per (dest, tensor) slot
    kv_in = nc.dram_tensor("kv_in", [NC, 2, SLOT], F16)
    kv_out = nc.dram_tensor("kv_out", [NC, 2, SLOT], F16)
    q_in = nc.dram_tensor("q_in", [NC, SLOT], F16)
    q_out = nc.dram_tensor("q_out", [NC, SLOT], F16)
    cc_in = nc.dram_tensor("cc_in", [NC, SLOT], F16)
    cc_out = nc.dram_tensor("cc_out", [NC, SLOT], F16)

    from contextlib import ExitStack
    with tile.TileContext(nc) as tc, ExitStack() as es:
        # ---------------- pools ----------------
        const = es.enter_context(tc.tile_pool(name="const", bufs=1))
        persist = es.enter_context(tc.tile_pool(name="persist", bufs=1))
        wpool = es.enter_context(tc.tile_pool(name="wpool", bufs=6))  # f32 [128,512]
        wpool16 = es.enter_context(tc.tile_pool(name="wpool16", bufs=4))  # f16 weight tiles
        sc_pool = es.enter_context(tc.tile_pool(name="scratch", bufs=3))
        small = es.enter_context(tc.tile_pool(name="small", bufs=8))
        pacc = es.enter_context(tc.tile_pool(name="pacc", bufs=1, space="PSUM"))  # 4 acc tags = 4 banks
        pmix = es.enter_context(tc.tile_pool(name="pmix", bufs=4, space="PSUM"))  # shared rotating tag = 3 banks

        ident = const.tile([128, 128], F32)
        make_identity(nc, ident)
        tri = const.tile([128, 2, 256], F32)
        nc.sync.dma_start(out=tri[:], in_=tri_io.ap().rearrange("a p q -> p a q"))
        bqk_col = const.tile([128, 2 * KT], F32)
        nc.sync.dma_start(out=bqk_col[:], in_=bqk_col_io.ap())
        ff1b_col = const.tile([128, FF // 128], F32)
        nc.sync.dma_start(out=ff1b_col[:], in_=ff1b_col_io.ap())
        eps_col = const.tile([128, 1], F32)
        nc.vector.memset(eps_col[:], EPS)
        ones64 = const.tile([1, 64], F32)
        nc.vector.memset(ones64[:], 1.0)

        _round = [0]
        def acc_tiles():
            r = _round[0]; _round[0] += 1
            if r % 2 == 0:
                return [pacc.tile([128, 512], F32, tag=f"acc{t}", name=f"acc{t}")
                        for t in range(4)]
            return [pmix.tile([128, 512], F32, tag="pmix", name=f"accp{t}")
                    for t in range(4)]

        # residual stream, token-major [128, tt, H]
        x_sb = persist.tile([128, TT, H], F32)
        nc.sync.dma_start(out=x_sb[:], in_=x_io.ap().rearrange("(tt p) h -> p tt h", p=128))

        # slot-sharing tags: bigA = lnT (16K) then h (32K); bigB = qT (8K) then
        # ctxT (16K); bigC = kT then ctxo (8K); bigD = v then ln3T (8K)
        lnT_sb = persist.tile([128, KT, T], F32R, tag="bigA")
        qT_sb = persist.tile([128, KT, T], F16, tag="bigB")
        kT_sb = persist.tile([128, KT, T], F16, tag="bigC")
        v_sb = persist.tile([128, TT, H], F16, tag="bigD")
        g_bc = persist.tile([128, H], F32)              # gamma broadcast scratch
        beta_bc = persist.tile([128, H], F32)           # beta broadcast scratch
        bias_bc = persist.tile([128, H], F32)           # free-dim bias broadcast scratch

        row_pool = es.enter_context(tc.tile_pool(name="rows", bufs=1))

        def bcast_row(dst, src_dram, n):
            """Broadcast a [n] DRAM row across 128 partitions via zero-stride DMA."""
            src = src_dram.ap().unsqueeze(0).partition_broadcast(128).squeeze(1)
            nc.gpsimd.dma_start(out=dst[:, :n], in_=src)

        # ---------------- layernorm (token-major) + transpose ----------------
        def layer_norm_t(g_name, b_name, dst):
            """LN over x_sb tokens; writes transposed output into dst [128, kt, T]."""
            bcast_row(g_bc, gb_io[g_name], H)
            bcast_row(beta_bc, gb_io[b_name], H)
            for tt in range(TT):
                xt = x_sb[:, tt, :]
                sums = small.tile([128, 1], F32, tag="s0")
                sumsq = small.tile([128, 1], F32, tag="s1")
                lt = sc_pool.tile([128, H], F32, tag="lnt")
                nc.vector.reduce_sum(sums[:], xt, axis=mybir.AxisListType.X)
                nc.scalar.activation(lt[:], xt, AF.Square, accum_out=sumsq[:])
                mu = small.tile([128, 1], F32, tag="s2")
                var = small.tile([128, 1], F32, tag="s3")
                rstd = small.tile([128, 1], F32, tag="s4")
                nc.vector.tensor_scalar_mul(mu[:], sums[:], 1.0 / H)
                nc.vector.tensor_scalar_mul(var[:], sumsq[:], 1.0 / H)
                nc.vector.tensor_tensor(rstd[:], mu[:], mu[:], MUL)
                nc.vector.tensor_tensor(var[:], var[:], rstd[:], SUB)
                nc.scalar.activation(rstd[:], var[:], AF.Sqrt, bias=eps_col[:])
                nc.vector.reciprocal(rstd[:], rstd[:])
                nc.vector.tensor_scalar(lt[:], xt, mu[:], rstd[:], op0=SUB, op1=MUL)
                nc.any.tensor_mul(lt[:], lt[:], g_bc[:])
                nc.any.tensor_add(lt[:], lt[:], beta_bc[:])
                for kt in range(KT):
                    pt = pmix.tile([128, 512], F32, tag="pmix", name="pt")
                    nc.tensor.transpose(pt[:, :128], lt[:, bass.ts(kt, 128)], ident[:])
                    nc.any.tensor_copy(dst[:, kt, bass.ts(tt, 128)], pt[:, :128])

        # =====================================================================
        # Stage 1: x += LN1(x) @ (sf_w * mask).T + sf_b
        # =====================================================================
        layer_norm_t("g1", "b1", lnT_sb)
        bcast_row(bias_bc, b_io["sf_b"], H)
        for nch in range(2):
            ps = acc_tiles()
            for kt in range(KT):
                wt = wpool.tile([128, 512], F32R, tag="wa")
                mt = wpool.tile([128, 512], F32, tag="wb")
                nc.sync.dma_start(out=wt[:], in_=_r(sfwT_io.ap()[bass.ts(kt, 128), bass.ts(nch, 512)]))
                nc.sync.dma_start(out=mt[:], in_=maskT_io.ap()[bass.ts(kt, 128), bass.ts(nch, 512)])
                nc.any.tensor_mul(wt[:], wt[:], mt[:])
                for tt in range(TT):
                    nc.tensor.matmul(ps[tt][:], lnT_sb[:, kt, bass.ts(tt, 128)],
                                     wt[:], start=(kt == 0), stop=(kt == KT - 1))
            for tt in range(TT):
                xsl = x_sb[:, tt, bass.ts(nch, 512)]
                tmp = sc_pool.tile([128, 512], F32, tag="ev")
                nc.any.tensor_add(tmp[:], ps[tt][:], bias_bc[:, bass.ts(nch, 512)])
                nc.any.tensor_add(xsl, xsl, tmp[:])

        # =====================================================================
        # Stage 2: LN2 + QKV
        # =====================================================================
        layer_norm_t("g2", "b2", lnT_sb)
        # q/k: feature-major out [n 128, t 512]; scale q/k by 1/sqrt(sqrt(D)) each
        # so scores come out pre-scaled by 1/sqrt(D).
        qsc = 1.0 / float(np.sqrt(np.sqrt(D)))
        for wio, dst, bcol in [("wkT", kT_sb, 1)]:
            for nh in range(2):
                ps = acc_tiles()
                for kt in range(KT):
                    wt = wpool.tile([128, 512], F32R, tag="wa")
                    nc.sync.dma_start(out=wt[:], in_=_r(wT_io[wio].ap()[bass.ts(kt, 128), bass.ts(nh, 512)]))
                    for n4 in range(4):
                        nc.tensor.matmul(ps[n4][:], wt[:, bass.ts(n4, 128)], lnT_sb[:, kt, :],
                                         start=(kt == 0), stop=(kt == KT - 1))
                for n4 in range(4):
                    nt = nh * 4 + n4
                    nc.any.tensor_scalar(dst[:, nt, :], ps[n4][:],
                                         bqk_col[:, bcol * KT + nt: bcol * KT + nt + 1],
                                         qsc, op0=ADD, op1=MUL)
        # v: token-major out [t 128, n 512]
        bcast_row(bias_bc, b_io["bv"], H)
        for nch in range(2):
            ps = acc_tiles()
            for kt in range(KT):
                wt = wpool.tile([128, 512], F32R, tag="wa")
                nc.sync.dma_start(out=wt[:], in_=_r(wT_io["wvT"].ap()[bass.ts(kt, 128), bass.ts(nch, 512)]))
                for tt in range(TT):
                    nc.tensor.matmul(ps[tt][:], lnT_sb[:, kt, bass.ts(tt, 128)],
                                     wt[:], start=(kt == 0), stop=(kt == KT - 1))
            for tt in range(TT):
                nc.any.tensor_add(v_sb[:, tt, bass.ts(nch, 512)], ps[tt][:],
                                  bias_bc[:, bass.ts(nch, 512)])

        # =====================================================================
        # A2A #1a: k/v exchange (overlaps with q production below)
        # =====================================================================
        for j in range(NC):
            nc.sync.dma_start(out=kv_in.ap()[j, 0].rearrange("(p t) -> p t", p=128),
                              in_=kT_sb[:, j, :])
            nc.sync.dma_start(out=kv_in.ap()[j, 1].rearrange("(p tt f) -> p tt f", p=128, tt=TT),
                              in_=v_sb[:, :, bass.ts(j, 128)])
        nc.gpsimd.collective_compute(
            "AllToAll", mybir.AluOpType.bypass, replica_groups=RG,
            ins=[kv_in.ap().opt()], outs=[kv_out.ap().opt()])
        # q production (overlaps the kv A2A)
        for wio, dst, bcol in [("wqT", qT_sb, 0)]:
            for nh in range(2):
                ps = acc_tiles()
                for kt in range(KT):
                    wt = wpool.tile([128, 512], F32R, tag="wa")
                    nc.sync.dma_start(out=wt[:], in_=_r(wT_io[wio].ap()[bass.ts(kt, 128), bass.ts(nh, 512)]))
                    for n4 in range(4):
                        nc.tensor.matmul(ps[n4][:], wt[:, bass.ts(n4, 128)], lnT_sb[:, kt, :],
                                         start=(kt == 0), stop=(kt == KT - 1))
                for n4 in range(4):
                    nt = nh * 4 + n4
                    nc.any.tensor_scalar(dst[:, nt, :], ps[n4][:],
                                         bqk_col[:, bcol * KT + nt: bcol * KT + nt + 1],
                                         qsc, op0=ADD, op1=MUL)
        for j in range(NC):
            nc.sync.dma_start(out=q_in.ap()[j].rearrange("(p t) -> p t", p=128),
                              in_=qT_sb[:, j, :])
        nc.gpsimd.collective_compute(
            "AllToAll", mybir.AluOpType.bypass, replica_groups=RG,
            ins=[q_in.ap().opt()], outs=[q_out.ap().opt()])

        # =====================================================================
        # Attention: 2 heads, full sequence, exact causal
        # =====================================================================
        ctxT_sb = persist.tile([128, B, S], F16, tag="bigB", name="ctxT_sb")
        att_pool = es.enter_context(tc.tile_pool(name="attp", bufs=4))
        qk_pool = es.enter_context(tc.tile_pool(name="qkp", bufs=2))
        vb_pool = es.enter_context(tc.tile_pool(name="vbp", bufs=2))
        rr_pool = es.enter_context(tc.tile_pool(name="rrp", bufs=2))
        SKT = S // 128   # 16 kk tiles per batch
        for b in range(B):
            # v for this batch: [128, src(4), tt(4), h(2), 65] fp16 (65th col = 1)
            vb = vb_pool.tile([128, 4, TT, HPC, D + 1], F16, tag="vb")
            for i in range(4):
                src = 4 * b + i
                nc.sync.dma_start(
                    out=vb[:, i, :, :, 0:D],
                    in_=kv_out.ap()[src, 1].rearrange("(p tt h d) -> p tt h d",
                                                       p=128, tt=TT, h=HPC))
            nc.vector.memset(vb[:, :, :, :, D:D + 1], 1.0)
            for h in range(HPC):
                qa = qk_pool.tile([64, 4, T], F16, tag="qa")
                ka = qk_pool.tile([64, 4, T], F16, tag="ka")
                for i in range(4):
                    src = 4 * b + i
                    nc.sync.dma_start(
                        out=qa[:, i, :],
                        in_=q_out.ap()[src].rearrange("(p t) -> p t", p=128)[bass.ts(h, 64), :])
                    nc.sync.dma_start(
                        out=ka[:, i, :],
                        in_=kv_out.ap()[src, 0].rearrange("(p t) -> p t", p=128)[bass.ts(h, 64), :])
                qf = qa[:].rearrange("p a t -> p (a t)")
                kf = ka[:].rearrange("p a t -> p (a t)")
                for qp in range(S // 256):
                    nkt = 2 * qp + 2
                    cx = pmix.tile([65, 256], F32, tag="pmix", name="cx")
                    for kt in range(nkt):
                        if kt % 2 == 0:
                            sc = pacc.tile([128, 512], F32, tag=f"acc{kt % 4}", name="sc")
                            sc = sc[:, :256]
                        else:
                            sc = pmix.tile([128, 256], F32, tag="pmix", name="sc")
                        att = att_pool.tile([128, 256], F16, tag="att")
                        nc.tensor.matmul(sc[:], kf[:, bass.ts(kt, 128)],
                                         qf[:, bass.ts(qp, 256)], start=True, stop=True)
                        if kt < 2 * qp:
                            nc.any.tensor_scalar_max(att[:], sc[:], 0.0)
                        else:  # diagonal tiles: mask then relu
                            nc.any.tensor_mul(att[:], sc[:], tri[:, kt - 2 * qp, :])
                            nc.any.tensor_scalar_max(att[:], att[:], 0.0)
                        nc.tensor.matmul(cx[:], vb[:, kt // TT, kt % TT, h, :],
                                         att[:], start=(kt == 0), stop=(kt == nkt - 1))
                    # normalize: ctxT[d, q] * 1/(rowsum[q] + 1e-9)
                    rs = rr_pool.tile([1, 256], F32, tag="rs")
                    rb = rr_pool.tile([64, 256], F32, tag="rb")
                    nc.vector.tensor_scalar_add(rs[:], cx[64:65, :], 1e-9)
                    nc.vector.reciprocal(rs[:], rs[:])
                    rbp = pmix.tile([64, 256], F32, tag="pmix", name="rbp")
                    nc.tensor.matmul(rbp[:], ones64[:1, :], rs[:1, :], start=True, stop=True)
                    nc.vector.tensor_copy(rb[:], rbp[:])
                    nc.vector.tensor_tensor(
                        ctxT_sb[bass.ts(h, 64), b, bass.ts(qp, 256)],
                        cx[0:64, :], rb[:], MUL)

        # =====================================================================
        # A2A #2: head-sharded ctx -> token-sharded
        # =====================================================================
        for j in range(NC):
            nc.sync.dma_start(out=cc_in.ap()[j].rearrange("(p t) -> p t", p=128),
                              in_=ctxT_sb[:, :, :].rearrange("p b s -> p (b s)")[:, bass.ds(j * T, T)])
        nc.gpsimd.collective_compute(
            "AllToAll", mybir.AluOpType.bypass, replica_groups=RG,
            ins=[cc_in.ap().opt()], outs=[cc_out.ap().opt()])
        ctxo_sb = persist.tile([128, KT, T], F16, tag="bigC", name="ctxo_sb")
        for j in range(NC):
            nc.sync.dma_start(out=ctxo_sb[:, j, :],
                              in_=cc_out.ap()[j].rearrange("(p t) -> p t", p=128))
        ctxo32 = persist.tile([128, KT, T], F32R, tag="bigB", name="ctxo32")
        for j in range(NC):
            nc.any.tensor_copy(ctxo32[:, j, :], ctxo_sb[:, j, :])

        # =====================================================================
        # out-proj: x += ctx @ wo.T + bo  (fp16)
        # =====================================================================
        bcast_row(bias_bc, b_io["bo"], H)
        for nch in range(2):
            ps = acc_tiles()
            for kt in range(KT):
                wt = wpool.tile([128, 512], F32R, tag="wa")
                nc.sync.dma_start(out=wt[:], in_=_r(wT_io["woT"].ap()[bass.ts(kt, 128), bass.ts(nch, 512)]))
                for tt in range(TT):
                    nc.tensor.matmul(ps[tt][:], ctxo32[:, kt, bass.ts(tt, 128)],
                                     wt[:], start=(kt == 0), stop=(kt == KT - 1))
            for tt in range(TT):
                xsl = x_sb[:, tt, bass.ts(nch, 512)]
                tmp = sc_pool.tile([128, 512], F32, tag="ev")
                nc.any.tensor_add(tmp[:], ps[tt][:], bias_bc[:, bass.ts(nch, 512)])
                nc.any.tensor_add(xsl, xsl, tmp[:])

        # =====================================================================
        # FFN (fp16): x += relu(LN3(x) @ w1.T + b1f) @ w2.T + b2f
        # =====================================================================
        ln3T_sb = persist.tile([128, KT, T], F32R, tag="bigD", name="ln3T_sb")
        layer_norm_t("g3", "b3", ln3T_sb)
        h_sb = persist.tile([128, FF // 128, T], F16, tag="bigA", name="h_sb")
        NFT = FF // 128  # 32
        for nh in range(NFT // 4):
            ps = acc_tiles()
            for kt in range(KT):
                wt = wpool.tile([128, 512], F32R, tag="wa")
                nc.sync.dma_start(out=wt[:], in_=_r(w1T_io.ap()[bass.ts(kt, 128), bass.ts(nh, 512)]))
                for n4 in range(4):
                    nc.tensor.matmul(ps[n4][:], wt[:, bass.ts(n4, 128)], ln3T_sb[:, kt, :],
                                     start=(kt == 0), stop=(kt == KT - 1))
            for n4 in range(4):
                nt = nh * 4 + n4
                nc.scalar.activation(h_sb[:, nt, :], ps[n4][:], AF.Relu,
                                     bias=ff1b_col[:, nt:nt + 1])
        bcast_row(bias_bc, b_io["ff2_b"], H)
        for nch in range(2):
            ps = acc_tiles()
            for kt in range(NFT):
                wf = wpool.tile([128, 512], F32, tag="wb")
                nc.sync.dma_start(out=wf[:], in_=w2T_io.ap()[bass.ts(kt, 128), bass.ts(nch, 512)])
                wt = wpool16.tile([128, 512], F16, tag="w16")
                nc.any.tensor_copy(wt[:], wf[:])
                for tt in range(TT):
                    nc.tensor.matmul(ps[tt][:], h_sb[:, kt, bass.ts(tt, 128)],
                                     wt[:], start=(kt == 0), stop=(kt == NFT - 1))
            for tt in range(TT):
                xsl = x_sb[:, tt, bass.ts(nch, 512)]
                tmp = sc_pool.tile([128, 512], F32, tag="ev")
                nc.any.tensor_add(tmp[:], ps[tt][:], bias_bc[:, bass.ts(nch, 512)])
                nc.any.tensor_add(xsl, xsl, tmp[:])

        # final output
        nc.sync.dma_start(out=out_io.ap().rearrange("(tt p) h -> p tt h", p=128),
                          in_=x_sb[:])

    nc.compile()
    return nc


def _prep_shared(inputs):
    f = lambda a: np.ascontiguousarray(np.asarray(a, np.float32))
    sh = {
        "sfwT": f(inputs["sf_w"]).T.copy(),
        "maskT": f(inputs["mask"]).T.copy(),
        "wqT": f(inputs["wq"]).T.copy(),
        "wkT": f(inputs["wk"]).T.copy(),
        "wvT": f(inputs["wv"]).T.copy(),
        "woT": f(inputs["wo"]).T.copy(),
        "w1T": f(inputs["ff1_w"]).T.copy(),
        "w2T": f(inputs["ff2_w"]).T.copy(),
        "ff1_b": f(inputs["ff1_b"]),
    }
    for k in ("sf_b", "bq", "bk", "bv", "bo"):
        sh[k] = f(inputs[k])
    sh["ff2_b"] = f(inputs["ff2_b"])
    for k in ("g1", "b1", "g2", "b2", "g3", "b3"):
        sh[k] = f(inputs[k])
    # diag masks: tri[0] = [tril.T | ones], tri[1] = [zeros | tril.T]
    tri = np.zeros((2, 128, 256), np.float32)
    tl = np.tril(np.ones((128, 128), np.float32)).T  # valid: kk(row) <= q(col)
    tri[0, :, :128] = tl
    tri[0, :, 128:] = 1.0
    tri[1, :, 128:] = tl
    sh["tri"] = tri
    sh["bqk_col"] = np.stack([sh["bq"], sh["bk"]]).reshape(2 * KT, 128).T.copy().reshape(128, 2 * KT)
    sh["ff1b_col"] = sh["ff1_b"].reshape(FF // 128, 128).T.copy()
    return sh


def kernel(**inputs) -> np.ndarray:
    from concourse.bass_utils import run_bass_kernel_spmd

    if "nc" not in _CACHE:
        _CACHE["nc"] = _build()
    nc = _CACHE["nc"]

    sh = _prep_shared(inputs)
    x = np.ascontiguousarray(np.asarray(inputs["x"], np.float32)).reshape(B * S, H)
    in_maps = []
    for c in range(NC):
        m = dict(sh)
        m["x_c"] = np.ascontiguousarray(x[c * T:(c + 1) * T])
        in_maps.append(m)

    res = run_bass_kernel_spmd(nc, in_maps, core_ids=list(range(NC)))
    out = np.concatenate([res.results[c]["out_c"] for c in range(NC)], axis=0)
    return out.reshape(B, S, H).astype(np.float32)



# revision 10
# speedup vs baseline: 1.3576x; 1.3576x over previous
# Trainium2 Bass kernel for nn_BDHBlock (dense transformer block).
#
# Strategy (8 NeuronCores, one shared SPMD program):
#   - Token-parallel for token-local stages: core c owns flat tokens
#     [512c, 512c+512) of x.reshape(4096, 1024). LayerNorms, the masked
#     sparse linear, QKV / output projections and the FFN run locally with
#     replicated weights (host pre-transposed, pre-masked, cast to fp16).
#   - Attention is head-parallel: AllToAll reshards q/k/v from token-sharded
#     to head-sharded (2 heads x full 4096-token sequence per core), each
#     core runs exact-causal relu attention for its 2 heads, and a second
#     AllToAll reshards the (unnormalized) context + row-sums back to
#     token-sharded, where the 1/(rowsum+eps) scaling is applied.
#   - All matmuls in fp16 (full-rate) with fp32 PSUM accumulation; the
#     fp32 residual stream stays in SBUF.
#   - Score matmuls for the two heads are row-tiled (K=64 each at array
#     rows 0-63 / 64-127) so they can run concurrently in the PE array.
import numpy as np

import concourse.bass as bass
import concourse.mybir as mybir
import concourse.tile as tile
from concourse import bacc
from concourse.masks import make_identity

B, S, H, NH = 2, 2048, 1024, 16
D = H // NH            # 64
FF = 4 * H             # 4096
NC = 8                 # cores
T = B * S // NC        # 512 tokens per core
TT = T // 128          # 4 token tiles
KT = H // 128          # 8 feature tiles
NFT = FF // 128        # 32
HPC = 2                # heads per core
SLOT = 128 * T         # elements per (dest, tensor) A2A slot
CSLOT = SLOT + 2 * T   # ctx slot + rowsum tail (2 heads x T tokens)
F32, F32R, F16 = mybir.dt.float32, mybir.dt.float32r, mybir.dt.float16
ADD, SUB, MUL, MAX = (mybir.AluOpType.add, mybir.AluOpType.subtract,
                      mybir.AluOpType.mult, mybir.AluOpType.max)
AF = mybir.ActivationFunctionType
RG = [list(range(NC))]
EPS = 1e-5

_CACHE = {}


def _r(ap):
    return ap.bitcast(F32R)


def _build():
    nc = bacc.Bacc("TRN2", target_bir_lowering=False, debug=False,
                   num_devices=NC)

    # ---------------- I/O ----------------
    def inp(name, shape, dtype=F32):
        return nc.dram_tensor(name, list(shape), dtype, kind="ExternalInput")

    x_io = inp("x_c", (T, H))
    sfwT_io = inp("sfwT", (H, H), F16)          # (sf_w * mask).T
    wT_io = {k: inp(k, (H, H), F16) for k in ("wqT", "wkT", "wvT", "woT")}
    w1T_io = inp("w1T", (H, FF), F16)
    w2T_io = inp("w2T", (FF, H), F16)
    biasrow_io = inp("biasrow", (1, 4 * H), F16)   # sf_b | bv? | bo | ff2_b
    bqkv_col_io = inp("bqkv_col", (128, 3 * KT))   # bq,bk (pre-scaled), bv
    ff1b_col_io = inp("ff1b_col", (128, NFT))
    gbT_io = inp("gbT", (128, 6 * KT))             # g1 b1 g2 b2 g3 b3 columns
    tri2_io = inp("tri2", (128, 512), F16)         # fused diag masks
    rsel_io = inp("rsel", (16, KT * 128))          # head-expand selector
    out_io = nc.dram_tensor("out_c", [T, H], F32, kind="ExternalOutput")

    # internal DRAM for collectives (HBM bounce)
    kv_in = nc.dram_tensor("kv_in", [NC, 2, SLOT], F16)
    kv_out = nc.dram_tensor("kv_out", [NC, 2, SLOT], F16)
    q_in = nc.dram_tensor("q_in", [NC, SLOT], F16)
    q_out = nc.dram_tensor("q_out", [NC, SLOT], F16)
    cc_in = nc.dram_tensor("cc_in", [NC, CSLOT], F16)
    cc_out = nc.dram_tensor("cc_out", [NC, CSLOT], F16)

    from contextlib import ExitStack
    with tile.TileContext(nc) as tc, ExitStack() as es:
        # ---------------- pools ----------------
        const = es.enter_context(tc.tile_pool(name="const", bufs=1))
        persist = es.enter_context(tc.tile_pool(name="persist", bufs=1))
        wpool = es.enter_context(tc.tile_pool(name="wpool", bufs=8))
        w12pool = es.enter_context(tc.tile_pool(name="w12pool", bufs=8))
        sc_pool = es.enter_context(tc.tile_pool(name="scratch", bufs=2))
        small = es.enter_context(tc.tile_pool(name="small", bufs=8))
        attp = es.enter_context(tc.tile_pool(name="attp", bufs=2))
        att_sb = es.enter_context(tc.tile_pool(name="att_sb", bufs=4))
        pacc = es.enter_context(tc.tile_pool(name="pacc", bufs=1, space="PSUM"))
        pmix = es.enter_context(tc.tile_pool(name="pmix", bufs=4, space="PSUM"))

        ident = const.tile([128, 128], F16)
        make_identity(nc, ident)
        tri2 = const.tile([128, 512], F16)
        nc.sync.dma_start(out=tri2[:], in_=tri2_io.ap())
        ones512 = const.tile([1, 512], F16)
        nc.vector.memset(ones512[:], 1.0)
        bqkv_col = const.tile([128, 3 * KT], F32)
        nc.sync.dma_start(out=bqkv_col[:], in_=bqkv_col_io.ap())
        ff1b_col = const.tile([128, NFT], F32)
        nc.sync.dma_start(out=ff1b_col[:], in_=ff1b_col_io.ap())
        gbT = const.tile([128, 6 * KT], F32)
        nc.sync.dma_start(out=gbT[:], in_=gbT_io.ap())
        biasrow = const.tile([1, 4 * H], F16)
        nc.sync.dma_start(out=biasrow[:], in_=biasrow_io.ap())
        rsel = const.tile([16, KT * 128], F32)
        nc.sync.dma_start(out=rsel[:], in_=rsel_io.ap())
        eps_col = const.tile([128, 1], F32)
        nc.vector.memset(eps_col[:], EPS)

        _round = [0]

        def acc_tiles():
            r = _round[0]
            _round[0] += 1
            if r % 2 == 0:
                return [pacc.tile([128, 512], F32, tag=f"acc{t}", name=f"acc{t}")
                        for t in range(4)]
            return [pmix.tile([128, 512], F32, tag="pmix", name=f"accp{t}")
                    for t in range(4)]

        # residual stream, token-major [128, tt, H] fp32
        x_sb = persist.tile([128, TT, H], F32)
        for tt in range(TT):
            nc.sync.dma_start(
                out=x_sb[:, tt, :],
                in_=x_io.ap().rearrange("(tt p) h -> p tt h", p=128)[:, tt, :])

        ln_a = persist.tile([128, KT, T], F16, name="ln_a")   # LN1 / LN3 out^T
        ln_b = persist.tile([128, KT, T], F16, name="ln_b")   # LN2 out^T
        kT_sb = persist.tile([128, KT, T], F16, name="kT_sb")
        vT_sb = persist.tile([128, KT, T], F16, name="vT_sb")
        qT_sb = persist.tile([128, KT, T], F16, name="qT_sb")

        # ---------------- layernorm (token-major) + transpose ----------------
        def layer_norm_t(li, dst):
            """LN over x_sb tokens; transposed fp16 output with g/b applied."""
            gcol0 = (2 * li) * KT
            bcol0 = (2 * li + 1) * KT
            for tt in range(TT):
                xt = x_sb[:, tt, :]
                sums = small.tile([128, 1], F32, tag="s0")
                sumsq = small.tile([128, 1], F32, tag="s1")
                sq = sc_pool.tile([128, H], F32, tag="lnsq")
                nc.vector.reduce_sum(sums[:], xt, axis=mybir.AxisListType.X)
                nc.scalar.activation(sq[:], xt, AF.Square, accum_out=sumsq[:])
                mu = small.tile([128, 1], F32, tag="s2")
                var = small.tile([128, 1], F32, tag="s3")
                rstd = small.tile([128, 1], F32, tag="s4")
                nc.vector.tensor_scalar_mul(mu[:], sums[:], 1.0 / H)
                nc.vector.tensor_scalar_mul(var[:], sumsq[:], 1.0 / H)
                nc.vector.tensor_tensor(rstd[:], mu[:], mu[:], MUL)
                nc.vector.tensor_tensor(var[:], var[:], rstd[:], SUB)
                nc.scalar.activation(rstd[:], var[:], AF.Sqrt, bias=eps_col[:])
                nc.vector.reciprocal(rstd[:], rstd[:])
                lt = sc_pool.tile([128, H], F16, tag="lnt")
                nc.vector.tensor_scalar(lt[:], xt, mu[:], rstd[:], op0=SUB, op1=MUL)
                for kt in range(KT):
                    pt = pmix.tile([128, 128], F16, tag="pmix", name="pt")
                    nc.tensor.transpose(pt[:], lt[:, bass.ts(kt, 128)], ident[:])
                    nc.any.tensor_scalar(dst[:, kt, bass.ts(tt, 128)], pt[:],
                                         gbT[:, gcol0 + kt:gcol0 + kt + 1],
                                         gbT[:, bcol0 + kt:bcol0 + kt + 1],
                                         op0=MUL, op1=ADD)

        def token_major_linear(src, w_io, bias_off, evict):
            """x-layout output: for nch groups accumulate src^T @ w + bias."""
            for nch in range(2):
                ps = acc_tiles()
                for tt in range(TT):
                    nc.tensor.matmul(ps[tt][:], ones512[:, 0:128],
                                     biasrow[:, bias_off + 512 * nch:
                                             bias_off + 512 * nch + 512],
                                     start=True, stop=False)
                for kt in range(KT):
                    wt = wpool.tile([128, 512], F16, tag="wa")
                    nc.sync.dma_start(
                        out=wt[:],
                        in_=w_io.ap()[bass.ts(kt, 128), bass.ts(nch, 512)])
                    for tt in range(TT):
                        nc.tensor.matmul(ps[tt][:], src[:, kt, bass.ts(tt, 128)],
                                         wt[:], start=False, stop=(kt == KT - 1))
                for tt in range(TT):
                    evict(ps[tt], tt, nch)

        def feat_major_linear(src, w_io, dst, bcol0):
            """feature-major output [128, kt, T]; per-partition bias fused."""
            for nh in range(2):
                ps = acc_tiles()
                for kt in range(KT):
                    wt = wpool.tile([128, 512], F16, tag="wa")
                    nc.sync.dma_start(
                        out=wt[:],
                        in_=w_io.ap()[bass.ts(kt, 128), bass.ts(nh, 512)])
                    for n4 in range(4):
                        nc.tensor.matmul(ps[n4][:], wt[:, bass.ts(n4, 128)],
                                         src[:, kt, :],
                                         start=(kt == 0), stop=(kt == KT - 1))
                for n4 in range(4):
                    nt = nh * 4 + n4
                    nc.scalar.activation(dst[:, nt, :], ps[n4][:], AF.Identity,
                                         bias=bqkv_col[:, bcol0 + nt:bcol0 + nt + 1])

        def evict_residual(ps, tt, nch):
            xsl = x_sb[:, tt, bass.ts(nch, 512)]
            nc.any.tensor_add(xsl, xsl, ps[:])

        # =====================================================================
        # Stage 1: x += LN1(x) @ (sf_w * mask).T + sf_b
        # =====================================================================
        with nc.named_scope("ln1"):
            layer_norm_t(0, ln_a)
        with nc.named_scope("stage1"):
            token_major_linear(ln_a, sfwT_io, 0 * H, evict_residual)

        # =====================================================================
        # Stage 2: LN2 + QKV (k, v feature-major; q feature-major)
        # =====================================================================
        with nc.named_scope("ln2"):
            layer_norm_t(1, ln_b)
        with nc.named_scope("kvproj"):
            feat_major_linear(ln_b, wT_io["wkT"], kT_sb, KT)       # bk col block
            feat_major_linear(ln_b, wT_io["wvT"], vT_sb, 2 * KT)   # bv col block

        # A2A #1: k/v exchange (overlaps q production below)
        with nc.named_scope("kvA2A"):
            for j in range(NC):
                nc.sync.dma_start(
                    out=kv_in.ap()[j, 0].rearrange("(p t) -> p t", p=128),
                    in_=kT_sb[:, j, :])
                nc.sync.dma_start(
                    out=kv_in.ap()[j, 1].rearrange("(p t) -> p t", p=128),
                    in_=vT_sb[:, j, :])
            nc.gpsimd.collective_compute(
                "AllToAll", mybir.AluOpType.bypass, replica_groups=RG,
                ins=[kv_in.ap().opt()], outs=[kv_out.ap().opt()])
        with nc.named_scope("qproj"):
            feat_major_linear(ln_b, wT_io["wqT"], qT_sb, 0)
            for j in range(NC):
                nc.sync.dma_start(
                    out=q_in.ap()[j].rearrange("(p t) -> p t", p=128),
                    in_=qT_sb[:, j, :])
            nc.gpsimd.collective_compute(
                "AllToAll", mybir.AluOpType.bypass, replica_groups=RG,
                ins=[q_in.ap().opt()], outs=[q_out.ap().opt()])

        # =====================================================================
        # Attention: 2 heads x 2 batches, full sequence, exact causal relu
        # =====================================================================
        ctxT_sb = persist.tile([128, B, S], F16, name="ctxT_sb")
        # rowsums: head h of this core at partition 64*h (ACT-legal bases)
        rs2_sb = persist.tile([128, B, S], F16, name="rs2_sb")
        SKT = S // 128   # 16 key tiles per batch
        with nc.named_scope("attn"):
            for b in range(B):
                k2 = attp.tile([128, 4, T], F16, tag="k2")
                q2 = attp.tile([128, 4, T], F16, tag="q2")
                v2 = attp.tile([128, 4, T], F16, tag="v2")
                nc.sync.dma_start(
                    out=k2[:],
                    in_=kv_out.ap()[4 * b:4 * b + 4, 0].rearrange(
                        "s (p t) -> p s t", p=128))
                nc.sync.dma_start(
                    out=q2[:],
                    in_=q_out.ap()[4 * b:4 * b + 4].rearrange(
                        "s (p t) -> p s t", p=128))
                nc.sync.dma_start(
                    out=v2[:],
                    in_=kv_out.ap()[4 * b:4 * b + 4, 1].rearrange(
                        "s (p t) -> p s t", p=128))
                kf = k2[:].rearrange("p s t -> p (s t)")
                qf = q2[:].rearrange("p s t -> p (s t)")
                vf = v2[:].rearrange("p s t -> p (s t)")
                # v^T -> token-major [128 tok, (h0 d64 | 1 | h1 d64 | 1)]
                vt = attp.tile([128, SKT, 130], F16, tag="vt")
                nc.vector.memset(vt[:, :, 64:65], 1.0)
                nc.vector.memset(vt[:, :, 129:130], 1.0)
                for kt in range(SKT):
                    pv = pmix.tile([128, 128], F16, tag="pmix", name="pv")
                    nc.tensor.transpose(pv[:], vf[:, bass.ts(kt, 128)], ident[:])
                    nc.any.tensor_copy(vt[:, kt, 0:64], pv[:, 0:64])
                    nc.any.tensor_copy(vt[:, kt, 65:129], pv[:, 64:128])
                for qp in range(S // 256):
                    cx = [pacc.tile([65, 256], F32, tag=f"acc{(qp % 2) * 2 + h}",
                                    name=f"cx{h}") for h in range(2)]
                    for i in range(qp + 1):        # kt pairs
                        sp = [pmix.tile([128, 512], F32, tag="pmix", name=f"sp{h}")
                              for h in range(2)]
                        for u in range(2):
                            kt = 2 * i + u
                            for h in range(2):
                                nc.tensor.matmul(
                                    sp[h][:, bass.ts(u, 256)],
                                    kf[bass.ts(h, 64), bass.ts(kt, 128)],
                                    qf[bass.ts(h, 64), bass.ts(qp, 256)],
                                    start=True, stop=True)
                        att = [att_sb.tile([128, 512], F16, tag="att",
                                           name=f"att{h}") for h in range(2)]
                        for h in range(2):
                            if i < qp:
                                nc.any.tensor_scalar_max(att[h][:], sp[h][:], 0.0)
                            else:   # diagonal pair: mask then relu
                                nc.any.tensor_mul(att[h][:], sp[h][:], tri2[:])
                                nc.any.tensor_scalar_max(att[h][:], att[h][:], 0.0)
                        for u in range(2):
                            kt = 2 * i + u
                            for h in range(2):
                                nc.tensor.matmul(
                                    cx[h][:], vt[:, kt, bass.ds(65 * h, 65)],
                                    att[h][:, bass.ts(u, 256)],
                                    start=(kt == 0), stop=(kt == 2 * qp + 1))
                    for h in range(2):
                        nc.any.tensor_copy(
                            ctxT_sb[bass.ts(h, 64), b, bass.ts(qp, 256)],
                            cx[h][0:64, :])
                        nc.any.tensor_copy(
                            rs2_sb[64 * h:64 * h + 1, b, bass.ts(qp, 256)],
                            cx[h][64:65, :])

        # =====================================================================
        # A2A #2: head-sharded (ctx, rowsum) -> token-sharded
        # =====================================================================
        with nc.named_scope("ccA2A"):
            ctxf = ctxT_sb[:].rearrange("p b s -> p (b s)")
            for j in range(NC):
                nc.sync.dma_start(
                    out=cc_in.ap()[j, 0:SLOT].rearrange("(p t) -> p t", p=128),
                    in_=ctxf[:, bass.ds(j * T, T)])
                for h in range(2):
                    nc.sync.dma_start(
                        out=cc_in.ap()[j, SLOT + h * T:SLOT + (h + 1) * T]
                        .unsqueeze(0),
                        in_=rs2_sb[64 * h:64 * h + 1, :, :]
                        .rearrange("p b s -> p (b s)")[:, bass.ds(j * T, T)])
            nc.gpsimd.collective_compute(
                "AllToAll", mybir.AluOpType.bypass, replica_groups=RG,
                ins=[cc_in.ap().opt()], outs=[cc_out.ap().opt()])

        ctxo = persist.tile([128, KT, T], F16, name="ctxo")
        with nc.named_scope("ctxnorm"):
            nc.sync.dma_start(
                out=ctxo[:],
                in_=cc_out.ap()[:, 0:SLOT].rearrange("j (p t) -> p j t", p=128))
            rsT = persist.tile([16, T], F16, name="rsT")
            rsq = persist.tile([16, T], F32, name="rsq")
            for j in range(NC):
                nc.sync.dma_start(
                    out=rsT[2 * j:2 * j + 2, :],
                    in_=cc_out.ap()[j, SLOT:CSLOT].rearrange("(r t) -> r t", r=2))
            nc.vector.tensor_scalar_add(rsq[:], rsT[:], 1e-9)
            nc.vector.reciprocal(rsq[:], rsq[:])
            for j in range(KT):
                sp = pmix.tile([128, 512], F32, tag="pmix", name="rsp")
                nc.tensor.matmul(sp[:], rsel[:, bass.ts(j, 128)], rsq[:],
                                 start=True, stop=True)
                nc.vector.tensor_tensor(ctxo[:, j, :], ctxo[:, j, :], sp[:], MUL)

        # =====================================================================
        # out-proj: x += ctx @ wo.T + bo
        # =====================================================================
        with nc.named_scope("woproj"):
            token_major_linear(ctxo, wT_io["woT"], 2 * H, evict_residual)

        # =====================================================================
        # FFN: x += relu(LN3(x) @ w1.T + ff1_b) @ w2.T + ff2_b
        # =====================================================================
        with nc.named_scope("ln3"):
            layer_norm_t(2, ln_a)
        h_sb = persist.tile([128, NFT, T], F16, name="h_sb")
        with nc.named_scope("ffn1"):
            for nh in range(NFT // 4):
                ps = acc_tiles()
                for kt in range(KT):
                    wt = w12pool.tile([128, 512], F16, tag="w1")
                    nc.sync.dma_start(
                        out=wt[:],
                        in_=w1T_io.ap()[bass.ts(kt, 128), bass.ts(nh, 512)])
                    for n4 in range(4):
                        nc.tensor.matmul(ps[n4][:], wt[:, bass.ts(n4, 128)],
                                         ln_a[:, kt, :],
                                         start=(kt == 0), stop=(kt == KT - 1))
                for n4 in range(4):
                    nt = nh * 4 + n4
                    nc.scalar.activation(h_sb[:, nt, :], ps[n4][:], AF.Relu,
                                         bias=ff1b_col[:, nt:nt + 1])
        with nc.named_scope("ffn2"):
            for nch in range(2):
                ps = acc_tiles()
                for tt in range(TT):
                    nc.tensor.matmul(ps[tt][:], ones512[:, 0:128],
                                     biasrow[:, 3 * H + 512 * nch:
                                             3 * H + 512 * nch + 512],
                                     start=True, stop=False)
                for kt in range(NFT):
                    wt = w12pool.tile([128, 512], F16, tag="w2")
                    nc.sync.dma_start(
                        out=wt[:],
                        in_=w2T_io.ap()[bass.ts(kt, 128), bass.ts(nch, 512)])
                    for tt in range(TT):
                        nc.tensor.matmul(ps[tt][:], h_sb[:, kt, bass.ts(tt, 128)],
                                         wt[:], start=False, stop=(kt == NFT - 1))
                for tt in range(TT):
                    xsl = x_sb[:, tt, bass.ts(nch, 512)]
                    nc.any.tensor_add(xsl, xsl, ps[tt][:])
                    nc.sync.dma_start(
                        out=out_io.ap().rearrange("(tt p) h -> p tt h", p=128)
                        [:, tt, bass.ts(nch, 512)],
                        in_=xsl)

    nc.compile()
    return nc


def _prep_shared(inputs):
    f = lambda a: np.ascontiguousarray(np.asarray(a, np.float32))
    h = lambda a: np.ascontiguousarray(a.astype(np.float16))
    qsc = float(D) ** -0.25
    sh = {
        "sfwT": h((f(inputs["sf_w"]) * f(inputs["mask"])).T),
        "wqT": h((f(inputs["wq"]) * qsc).T),
        "wkT": h((f(inputs["wk"]) * qsc).T),
        "wvT": h(f(inputs["wv"]).T),
        "woT": h(f(inputs["wo"]).T),
        "w1T": h(f(inputs["ff1_w"]).T),
        "w2T": h(f(inputs["ff2_w"]).T),
    }
    sh["biasrow"] = h(np.concatenate(
        [f(inputs["sf_b"]), np.zeros(H, np.float32), f(inputs["bo"]),
         f(inputs["ff2_b"])]).reshape(1, 4 * H))
    bqkv = np.stack([f(inputs["bq"]) * qsc, f(inputs["bk"]) * qsc,
                     f(inputs["bv"])])
    sh["bqkv_col"] = np.ascontiguousarray(
        bqkv.reshape(3 * KT, 128).T.astype(np.float32))
    sh["ff1b_col"] = np.ascontiguousarray(
        f(inputs["ff1_b"]).reshape(NFT, 128).T)
    gb = np.stack([f(inputs[k]) for k in ("g1", "b1", "g2", "b2", "g3", "b3")])
    sh["gbT"] = np.ascontiguousarray(gb.reshape(6 * KT, 128).T)
    # diag masks for the (kt_even | kt_odd) paired layout
    tl = np.tril(np.ones((128, 128), np.float32)).T  # valid: key(row) <= q(col)
    tri2 = np.zeros((128, 512), np.float32)
    tri2[:, 0:128] = tl
    tri2[:, 128:256] = 1.0
    tri2[:, 384:512] = tl
    sh["tri2"] = h(tri2)
    rsel = np.zeros((16, KT * 128), np.float32)
    for j in range(KT):
        for hh in range(2):
            rsel[2 * j + hh, j * 128 + 64 * hh: j * 128 + 64 * hh + 64] = 1.0
    sh["rsel"] = rsel
    return sh


def kernel(**inputs) -> np.ndarray:
    from concourse.bass_utils import run_bass_kernel_spmd

    if "nc" not in _CACHE:
        _CACHE["nc"] = _build()
    nc = _CACHE["nc"]

    sh = _prep_shared(inputs)
    x = np.ascontiguousarray(np.asarray(inputs["x"], np.float32)).reshape(B * S, H)
    in_maps = []
    for c in range(NC):
        m = dict(sh)
        m["x_c"] = np.ascontiguousarray(x[c * T:(c + 1) * T])
        in_maps.append(m)

    res = run_bass_kernel_spmd(nc, in_maps, core_ids=list(range(NC)))
    out = np.concatenate([res.results[c]["out_c"] for c in range(NC)], axis=0)
    return out.reshape(B, S, H).astype(np.float32)


# revision 15
# speedup vs baseline: 1.5939x; 1.1741x over previous
# Trainium2 Bass kernel for nn_BDHBlock (dense transformer block).
#
# Strategy (8 NeuronCores, one shared SPMD program):
#   - Token-parallel for token-local stages: core c owns flat tokens
#     [512c, 512c+512) of x.reshape(4096, 1024). LayerNorms, the masked
#     sparse linear, QKV / output projections and the FFN run locally with
#     replicated weights (host pre-transposed, pre-masked, cast to fp16).
#   - Attention is head-parallel: AllToAll reshards q/k/v from token-sharded
#     to head-sharded (2 heads x full 4096-token sequence per core), each
#     core runs exact-causal relu attention for its 2 heads, and a second
#     AllToAll reshards the (unnormalized) context + row-sums back to
#     token-sharded, where the 1/(rowsum+eps) scaling is applied.
#   - All matmuls in fp16 (full-rate) with fp32 PSUM accumulation; the
#     fp32 residual stream stays in SBUF.
#   - Score matmuls for the two heads are row-tiled (K=64 each at array
#     rows 0-63 / 64-127) so they can run concurrently in the PE array.
import numpy as np

import concourse.bass as bass
import concourse.mybir as mybir
import concourse.tile as tile
from concourse import bacc
from concourse.masks import make_identity

B, S, H, NH = 2, 2048, 1024, 16
D = H // NH            # 64
FF = 4 * H             # 4096
NC = 8                 # cores
T = B * S // NC        # 512 tokens per core
TT = T // 128          # 4 token tiles
KT = H // 128          # 8 feature tiles
NFT = FF // 128        # 32
HPC = 2                # heads per core
SLOT = 128 * T         # elements per (dest, tensor) A2A slot
CSLOT = SLOT + 2 * T   # ctx slot + rowsum tail (2 heads x T tokens)
F32, F32R, F16 = mybir.dt.float32, mybir.dt.float32r, mybir.dt.float16
ADD, SUB, MUL, MAX = (mybir.AluOpType.add, mybir.AluOpType.subtract,
                      mybir.AluOpType.mult, mybir.AluOpType.max)
AF = mybir.ActivationFunctionType
RG = [list(range(NC))]
EPS = 1e-5

_CACHE = {}


def _r(ap):
    return ap.bitcast(F32R)


def _build():
    nc = bacc.Bacc("TRN2", target_bir_lowering=False, debug=False,
                   num_devices=NC)

    # ---------------- I/O ----------------
    def inp(name, shape, dtype=F32):
        return nc.dram_tensor(name, list(shape), dtype, kind="ExternalInput")

    x_io = inp("x_c", (T, H))
    sfwT_io = inp("sfwT", (H, H), F16)          # (sf_w * mask).T
    wT_io = {k: inp(k, (H, H), F16) for k in ("wqT", "wkT", "wvT", "woT")}
    w1T_io = inp("w1T", (H, FF), F16)
    w2T_io = inp("w2T", (FF, H), F16)
    biasrow_io = inp("biasrow", (1, 4 * H), F16)   # sf_b | bv? | bo | ff2_b
    bqkv_col_io = inp("bqkv_col", (128, 3 * KT))   # bq,bk (pre-scaled), bv
    ff1b_col_io = inp("ff1b_col", (128, NFT))
    gbT_io = inp("gbT", (128, 6 * KT))             # g1 b1 g2 b2 g3 b3 columns
    tri2_io = inp("tri2", (128, 512), F16)         # fused diag masks
    rsel_io = inp("rsel", (16, KT * 128), F16)     # head-expand selector
    out_io = nc.dram_tensor("out_c", [T, H], F32, kind="ExternalOutput")

    # internal DRAM for collectives (HBM bounce)
    k_in = nc.dram_tensor("k_in", [NC, SLOT], F16)
    k_out = nc.dram_tensor("k_out", [NC, SLOT], F16)
    v_in = nc.dram_tensor("v_in", [NC, SLOT], F16)
    v_out = nc.dram_tensor("v_out", [NC, SLOT], F16)
    q_in = nc.dram_tensor("q_in", [NC, SLOT], F16)
    q_out = nc.dram_tensor("q_out", [NC, SLOT], F16)
    cc_in = nc.dram_tensor("cc_in", [NC, CSLOT], F16)
    cc_out = nc.dram_tensor("cc_out", [NC, CSLOT], F16)

    from contextlib import ExitStack
    with tile.TileContext(nc) as tc, ExitStack() as es:
        # ---------------- pools ----------------
        const = es.enter_context(tc.tile_pool(name="const", bufs=1))
        persist = es.enter_context(tc.tile_pool(name="persist", bufs=1))
        wpool = es.enter_context(tc.tile_pool(name="wpool", bufs=8))
        w12pool = es.enter_context(tc.tile_pool(name="w12pool", bufs=8))
        sc_pool = es.enter_context(tc.tile_pool(name="scratch", bufs=2))
        small = es.enter_context(tc.tile_pool(name="small", bufs=8))
        attp = es.enter_context(tc.tile_pool(name="attp", bufs=2))
        att_sb = es.enter_context(tc.tile_pool(name="att_sb", bufs=4))
        pacc = es.enter_context(tc.tile_pool(name="pacc", bufs=1, space="PSUM"))
        pmix = es.enter_context(tc.tile_pool(name="pmix", bufs=4, space="PSUM"))

        ident = const.tile([128, 128], F16)
        make_identity(nc, ident)
        tri2 = const.tile([128, 512], F16)
        nc.sync.dma_start(out=tri2[:], in_=tri2_io.ap())
        ones512 = const.tile([1, 512], F16)
        nc.vector.memset(ones512[:], 1.0)
        bqkv_col = const.tile([128, 3 * KT], F32)
        nc.sync.dma_start(out=bqkv_col[:], in_=bqkv_col_io.ap())
        ff1b_col = const.tile([128, NFT], F32)
        nc.sync.dma_start(out=ff1b_col[:], in_=ff1b_col_io.ap())
        gbT = const.tile([128, 6 * KT], F32)
        nc.sync.dma_start(out=gbT[:], in_=gbT_io.ap())
        biasrow = const.tile([1, 4 * H], F16)
        nc.sync.dma_start(out=biasrow[:], in_=biasrow_io.ap())
        rsel = const.tile([16, KT * 128], F16)
        nc.sync.dma_start(out=rsel[:], in_=rsel_io.ap())
        eps_col = const.tile([128, 1], F32)
        nc.vector.memset(eps_col[:], EPS)

        _round = [0]

        def acc_tiles():
            r = _round[0]
            _round[0] += 1
            if r % 2 == 0:
                return [pacc.tile([128, 512], F32, tag=f"acc{t}", name=f"acc{t}")
                        for t in range(4)]
            return [pmix.tile([128, 512], F32, tag="pmix", name=f"accp{t}")
                    for t in range(4)]

        # residual stream, token-major [128, tt, H] fp32
        x_sb = persist.tile([128, TT, H], F32)
        for tt in range(TT):
            nc.sync.dma_start(
                out=x_sb[:, tt, :],
                in_=x_io.ap().rearrange("(tt p) h -> p tt h", p=128)[:, tt, :])

        ln_a = persist.tile([128, KT, T], F16, name="ln_a")   # LN1 / LN3 out^T
        ln_b = persist.tile([128, KT, T], F16, name="ln_b")   # LN2 out^T
        kT_sb = persist.tile([128, KT, T], F16, name="kT_sb")
        vT_sb = persist.tile([128, KT, T], F16, name="vT_sb")
        qT_sb = persist.tile([128, KT, T], F16, name="qT_sb")

        # ---------------- layernorm (token-major) + transpose ----------------
        def layer_norm_t(li, dst):
            """LN over x_sb tokens; transposed fp16 output with g/b applied."""
            gcol0 = (2 * li) * KT
            bcol0 = (2 * li + 1) * KT
            for tt in range(TT):
                xt = x_sb[:, tt, :]
                sums = small.tile([128, 1], F32, tag="s0")
                sumsq = small.tile([128, 1], F32, tag="s1")
                sq = sc_pool.tile([128, H], F32, tag="lnsq")
                nc.vector.reduce_sum(sums[:], xt, axis=mybir.AxisListType.X)
                nc.scalar.activation(sq[:], xt, AF.Square, accum_out=sumsq[:])
                mu = small.tile([128, 1], F32, tag="s2")
                var = small.tile([128, 1], F32, tag="s3")
                rstd = small.tile([128, 1], F32, tag="s4")
                nc.vector.tensor_scalar_mul(mu[:], sums[:], 1.0 / H)
                nc.vector.tensor_scalar_mul(var[:], sumsq[:], 1.0 / H)
                nc.vector.tensor_tensor(rstd[:], mu[:], mu[:], MUL)
                nc.vector.tensor_tensor(var[:], var[:], rstd[:], SUB)
                nc.scalar.activation(rstd[:], var[:], AF.Sqrt, bias=eps_col[:])
                nc.vector.reciprocal(rstd[:], rstd[:])
                lt = sc_pool.tile([128, H], F16, tag="lnt")
                nc.vector.tensor_scalar(lt[:], xt, mu[:], rstd[:], op0=SUB, op1=MUL)
                for kt in range(KT):
                    pt = pmix.tile([128, 128], F16, tag="pmix", name="pt")
                    nc.tensor.transpose(pt[:], lt[:, bass.ts(kt, 128)], ident[:])
                    nc.any.tensor_scalar(dst[:, kt, bass.ts(tt, 128)], pt[:],
                                         gbT[:, gcol0 + kt:gcol0 + kt + 1],
                                         gbT[:, bcol0 + kt:bcol0 + kt + 1],
                                         op0=MUL, op1=ADD)

        def token_major_linear(src, w_io, bias_off, evict):
            """x-layout output: for nch groups accumulate src^T @ w + bias."""
            for nch in range(2):
                ps = acc_tiles()
                for tt in range(TT):
                    nc.tensor.matmul(ps[tt][:], ones512[:, 0:128],
                                     biasrow[:, bias_off + 512 * nch:
                                             bias_off + 512 * nch + 512],
                                     start=True, stop=False)
                for kt in range(KT):
                    wt = wpool.tile([128, 512], F16, tag="wa")
                    nc.sync.dma_start(
                        out=wt[:],
                        in_=w_io.ap()[bass.ts(kt, 128), bass.ts(nch, 512)])
                    for tt in range(TT):
                        nc.tensor.matmul(ps[tt][:], src[:, kt, bass.ts(tt, 128)],
                                         wt[:], start=False, stop=(kt == KT - 1))
                for tt in range(TT):
                    evict(ps[tt], tt, nch)

        def feat_major_linear(src, w_io, dst, bcol0):
            """feature-major output [128, kt, T]; per-partition bias fused."""
            for nh in range(2):
                ps = acc_tiles()
                for kt in range(KT):
                    wt = wpool.tile([128, 512], F16, tag="wa")
                    nc.sync.dma_start(
                        out=wt[:],
                        in_=w_io.ap()[bass.ts(kt, 128), bass.ts(nh, 512)])
                    for n4 in range(4):
                        nc.tensor.matmul(ps[n4][:], wt[:, bass.ts(n4, 128)],
                                         src[:, kt, :],
                                         start=(kt == 0), stop=(kt == KT - 1))
                for n4 in range(4):
                    nt = nh * 4 + n4
                    nc.scalar.activation(dst[:, nt, :], ps[n4][:], AF.Identity,
                                         bias=bqkv_col[:, bcol0 + nt:bcol0 + nt + 1])

        def evict_residual(ps, tt, nch):
            xsl = x_sb[:, tt, bass.ts(nch, 512)]
            nc.any.tensor_add(xsl, xsl, ps[:])

        # =====================================================================
        # Stage 1: x += LN1(x) @ (sf_w * mask).T + sf_b
        # =====================================================================
        with nc.named_scope("ln1"):
            layer_norm_t(0, ln_a)
        with nc.named_scope("stage1"):
            token_major_linear(ln_a, sfwT_io, 0 * H, evict_residual)

        # =====================================================================
        # Stage 2: LN2 + QKV (k, v feature-major; q feature-major)
        # =====================================================================
        with nc.named_scope("ln2"):
            layer_norm_t(1, ln_b)
        def bounce_a2a(src_sb, buf_in, buf_out):
            for j in range(NC):
                nc.sync.dma_start(
                    out=buf_in.ap()[j].rearrange("(p t) -> p t", p=128),
                    in_=src_sb[:, j, :])
            nc.gpsimd.collective_compute(
                "AllToAll", mybir.AluOpType.bypass, replica_groups=RG,
                ins=[buf_in.ap().opt()], outs=[buf_out.ap().opt()])

        # pipelined per-tensor A2As: each launches right after its projection
        with nc.named_scope("kproj"):
            feat_major_linear(ln_b, wT_io["wkT"], kT_sb, KT)
            bounce_a2a(kT_sb, k_in, k_out)
        with nc.named_scope("vproj"):
            feat_major_linear(ln_b, wT_io["wvT"], vT_sb, 2 * KT)
            bounce_a2a(vT_sb, v_in, v_out)
        with nc.named_scope("qproj"):
            feat_major_linear(ln_b, wT_io["wqT"], qT_sb, 0)
            bounce_a2a(qT_sb, q_in, q_out)

        # =====================================================================
        # Attention: 2 heads x 2 batches, full sequence, exact causal relu
        # =====================================================================
        ctxT_sb = persist.tile([128, B, S], F16, name="ctxT_sb")
        # rowsums: head h of this core at partition 64*h (ACT-legal bases)
        rs2_sb = persist.tile([128, B, S], F16, name="rs2_sb")
        SKT = S // 128   # 16 key tiles per batch
        ctxf = ctxT_sb[:].rearrange("p b s -> p (b s)")

        def cc_bounce(j):
            nc.sync.dma_start(
                out=cc_in.ap()[j, 0:SLOT].rearrange("(p t) -> p t", p=128),
                in_=ctxf[:, bass.ds(j * T, T)])
            for h in range(2):
                nc.sync.dma_start(
                    out=cc_in.ap()[j, SLOT + h * T:SLOT + (h + 1) * T]
                    .unsqueeze(0),
                    in_=rs2_sb[64 * h:64 * h + 1, :, :]
                    .rearrange("p b s -> p (b s)")[:, bass.ds(j * T, T)])

        with nc.named_scope("attn"):
            kf, qf, vt = {}, {}, {}
            for b in range(B):
                k2 = attp.tile([128, 4, T], F16, tag="k2", name=f"k2_{b}")
                q2 = attp.tile([128, 4, T], F16, tag="q2", name=f"q2_{b}")
                v2 = attp.tile([128, 4, T], F16, tag="v2", name=f"v2_{b}")
                nc.sync.dma_start(
                    out=k2[:], in_=k_out.ap()[4 * b:4 * b + 4].rearrange(
                        "s (p t) -> p s t", p=128))
                nc.sync.dma_start(
                    out=q2[:], in_=q_out.ap()[4 * b:4 * b + 4].rearrange(
                        "s (p t) -> p s t", p=128))
                nc.sync.dma_start(
                    out=v2[:], in_=v_out.ap()[4 * b:4 * b + 4].rearrange(
                        "s (p t) -> p s t", p=128))
                kf[b] = k2[:].rearrange("p s t -> p (s t)")
                qf[b] = q2[:].rearrange("p s t -> p (s t)")
                vf = v2[:].rearrange("p s t -> p (s t)")
                # v^T -> token-major [128 tok, (h0 d64 | 1 | h1 d64 | 1)]
                vtb = attp.tile([128, SKT, 130], F16, tag="vt", name=f"vt{b}")
                nc.vector.memset(vtb[:, :, 64:65], 1.0)
                nc.vector.memset(vtb[:, :, 129:130], 1.0)
                for kt in range(SKT):
                    pv = pmix.tile([128, 128], F16, tag="pmix", name="pv")
                    nc.tensor.transpose(pv[:], vf[:, bass.ts(kt, 128)], ident[:])
                    nc.any.tensor_copy(vtb[:, kt, 0:64], pv[:, 0:64])
                    nc.any.tensor_copy(vtb[:, kt, 65:129], pv[:, 64:128])
                vt[b] = vtb
            # both batches interleaved: 4 independent (b, h) streams keep the
            # PE busy while relu runs on DVE/ACT
            for qp in range(S // 256):
                cx = {(b, h): pacc.tile([65, 256], F32, tag=f"acc{2 * b + h}",
                                        name=f"cx{b}{h}")
                      for b in range(B) for h in range(2)}
                for i in range(qp + 1):        # kt pairs
                    att = {}
                    for b in range(B):
                        sp = [pmix.tile([128, 512], F32, tag="pmix",
                                        name=f"sp{b}{h}") for h in range(2)]
                        for u in range(2):
                            kt = 2 * i + u
                            for h in range(2):
                                nc.tensor.matmul(
                                    sp[h][:, bass.ts(u, 256)],
                                    kf[b][bass.ts(h, 64), bass.ts(kt, 128)],
                                    qf[b][bass.ts(h, 64), bass.ts(qp, 256)],
                                    start=True, stop=True)
                        for h in range(2):
                            a = att_sb.tile([128, 512], F16, tag="att",
                                            name=f"att{b}{h}")
                            if i < qp:
                                nc.any.tensor_scalar_max(a[:], sp[h][:], 0.0)
                            else:   # diagonal pair: mask then relu
                                nc.any.tensor_mul(a[:], sp[h][:], tri2[:])
                                nc.any.tensor_scalar_max(a[:], a[:], 0.0)
                            att[b, h] = a
                    for b in range(B):
                        for u in range(2):
                            kt = 2 * i + u
                            for h in range(2):
                                nc.tensor.matmul(
                                    cx[b, h][:],
                                    vt[b][:, kt, bass.ds(65 * h, 65)],
                                    att[b, h][:, bass.ts(u, 256)],
                                    start=(kt == 0), stop=(kt == 2 * qp + 1))
                for b in range(B):
                    for h in range(2):
                        nc.any.tensor_copy(
                            ctxT_sb[bass.ts(h, 64), b, bass.ts(qp, 256)],
                            cx[b, h][0:64, :])
                        nc.any.tensor_copy(
                            rs2_sb[64 * h:64 * h + 1, b, bass.ts(qp, 256)],
                            cx[b, h][64:65, :])
                if qp % 2 == 1:
                    # dests whose token range [j*T,(j+1)*T) is now complete
                    m = (qp - 1) // 2
                    cc_bounce(m)
                    cc_bounce(4 + m)

        # =====================================================================
        # A2A #2: head-sharded (ctx, rowsum) -> token-sharded
        # =====================================================================
        with nc.named_scope("ccA2A"):
            nc.gpsimd.collective_compute(
                "AllToAll", mybir.AluOpType.bypass, replica_groups=RG,
                ins=[cc_in.ap().opt()], outs=[cc_out.ap().opt()])

        ctxo = persist.tile([128, KT, T], F16, name="ctxo")
        with nc.named_scope("ctxnorm"):
            rsT = persist.tile([16, T], F16, name="rsT")
            rsq = persist.tile([16, T], F32, name="rsq")
            for j in range(NC):
                nc.sync.dma_start(
                    out=rsT[2 * j:2 * j + 2, :],
                    in_=cc_out.ap()[j, SLOT:CSLOT].rearrange("(r t) -> r t", r=2))
            nc.sync.dma_start(
                out=ctxo[:],
                in_=cc_out.ap()[:, 0:SLOT].rearrange("j (p t) -> p j t", p=128))
            nc.vector.tensor_scalar_add(rsq[:], rsT[:], 1e-9)
            nc.vector.reciprocal(rsq[:], rsq[:])
            # fp16-safe: clamp (only relevant for exact-zero rowsums where
            # the ctx numerator is exactly zero anyway)
            rsq16 = persist.tile([16, T], F16, name="rsq16")
            nc.vector.tensor_scalar(rsq16[:], rsq[:], 60000.0, None,
                                    op0=mybir.AluOpType.min)
            for j in range(KT):
                sp = pmix.tile([128, 512], F32, tag="pmix", name="rsp")
                nc.tensor.matmul(sp[:], rsel[:, bass.ts(j, 128)], rsq16[:],
                                 start=True, stop=True)
                nc.vector.tensor_tensor(ctxo[:, j, :], ctxo[:, j, :], sp[:], MUL)

        # =====================================================================
        # out-proj: x += ctx @ wo.T + bo
        # =====================================================================
        with nc.named_scope("woproj"):
            token_major_linear(ctxo, wT_io["woT"], 2 * H, evict_residual)

        # =====================================================================
        # FFN: x += relu(LN3(x) @ w1.T + ff1_b) @ w2.T + ff2_b
        # =====================================================================
        with nc.named_scope("ln3"):
            layer_norm_t(2, ln_a)
        h_sb = persist.tile([128, NFT, T], F16, name="h_sb")
        with nc.named_scope("ffn1"):
            for nh in range(NFT // 4):
                ps = acc_tiles()
                for kt in range(KT):
                    wt = w12pool.tile([128, 512], F16, tag="w1")
                    nc.sync.dma_start(
                        out=wt[:],
                        in_=w1T_io.ap()[bass.ts(kt, 128), bass.ts(nh, 512)])
                    for n4 in range(4):
                        nc.tensor.matmul(ps[n4][:], wt[:, bass.ts(n4, 128)],
                                         ln_a[:, kt, :],
                                         start=(kt == 0), stop=(kt == KT - 1))
                for n4 in range(4):
                    nt = nh * 4 + n4
                    nc.scalar.activation(h_sb[:, nt, :], ps[n4][:], AF.Relu,
                                         bias=ff1b_col[:, nt:nt + 1])
        with nc.named_scope("ffn2"):
            for nch in range(2):
                ps = acc_tiles()
                for tt in range(TT):
                    nc.tensor.matmul(ps[tt][:], ones512[:, 0:128],
                                     biasrow[:, 3 * H + 512 * nch:
                                             3 * H + 512 * nch + 512],
                                     start=True, stop=False)
                for kt in range(NFT):
                    wt = w12pool.tile([128, 512], F16, tag="w2")
                    nc.sync.dma_start(
                        out=wt[:],
                        in_=w2T_io.ap()[bass.ts(kt, 128), bass.ts(nch, 512)])
                    for tt in range(TT):
                        nc.tensor.matmul(ps[tt][:], h_sb[:, kt, bass.ts(tt, 128)],
                                         wt[:], start=False, stop=(kt == NFT - 1))
                for tt in range(TT):
                    xsl = x_sb[:, tt, bass.ts(nch, 512)]
                    nc.any.tensor_add(xsl, xsl, ps[tt][:])
                    nc.sync.dma_start(
                        out=out_io.ap().rearrange("(tt p) h -> p tt h", p=128)
                        [:, tt, bass.ts(nch, 512)],
                        in_=xsl)

    nc.compile()
    return nc


def _prep_shared(inputs):
    f = lambda a: np.ascontiguousarray(np.asarray(a, np.float32))
    h = lambda a: np.ascontiguousarray(a.astype(np.float16))
    qsc = float(D) ** -0.25
    sh = {
        "sfwT": h((f(inputs["sf_w"]) * f(inputs["mask"])).T),
        "wqT": h((f(inputs["wq"]) * qsc).T),
        "wkT": h((f(inputs["wk"]) * qsc).T),
        "wvT": h(f(inputs["wv"]).T),
        "woT": h(f(inputs["wo"]).T),
        "w1T": h(f(inputs["ff1_w"]).T),
        "w2T": h(f(inputs["ff2_w"]).T),
    }
    sh["biasrow"] = h(np.concatenate(
        [f(inputs["sf_b"]), np.zeros(H, np.float32), f(inputs["bo"]),
         f(inputs["ff2_b"])]).reshape(1, 4 * H))
    bqkv = np.stack([f(inputs["bq"]) * qsc, f(inputs["bk"]) * qsc,
                     f(inputs["bv"])])
    sh["bqkv_col"] = np.ascontiguousarray(
        bqkv.reshape(3 * KT, 128).T.astype(np.float32))
    sh["ff1b_col"] = np.ascontiguousarray(
        f(inputs["ff1_b"]).reshape(NFT, 128).T)
    gb = np.stack([f(inputs[k]) for k in ("g1", "b1", "g2", "b2", "g3", "b3")])
    sh["gbT"] = np.ascontiguousarray(gb.reshape(6 * KT, 128).T)
    # diag masks for the (kt_even | kt_odd) paired layout
    tl = np.tril(np.ones((128, 128), np.float32)).T  # valid: key(row) <= q(col)
    tri2 = np.zeros((128, 512), np.float32)
    tri2[:, 0:128] = tl
    tri2[:, 128:256] = 1.0
    tri2[:, 384:512] = tl
    sh["tri2"] = h(tri2)
    rsel = np.zeros((16, KT * 128), np.float32)
    for j in range(KT):
        for hh in range(2):
            rsel[2 * j + hh, j * 128 + 64 * hh: j * 128 + 64 * hh + 64] = 1.0
    sh["rsel"] = rsel.astype(np.float16)
    return sh


def kernel(**inputs) -> np.ndarray:
    from concourse.bass_utils import run_bass_kernel_spmd

    if "nc" not in _CACHE:
        _CACHE["nc"] = _build()
    nc = _CACHE["nc"]

    sh = _prep_shared(inputs)
    x = np.ascontiguousarray(np.asarray(inputs["x"], np.float32)).reshape(B * S, H)
    in_maps = []
    for c in range(NC):
        m = dict(sh)
        m["x_c"] = np.ascontiguousarray(x[c * T:(c + 1) * T])
        in_maps.append(m)

    res = run_bass_kernel_spmd(nc, in_maps, core_ids=list(range(NC)))
    out = np.concatenate([res.results[c]["out_c"] for c in range(NC)], axis=0)
    return out.reshape(B, S, H).astype(np.float32)
